# revision 2
# baseline (speedup 1.0000x reference)
"""Block-sparse linear layer (x @ (mask*W).T + bias) on 8 TRN2 NeuronCores.

Strategy: data-parallel over batch rows. Each core gets 1024 rows of x
(transposed to [k, m] on host, cast to bf16), the packed kept weight
blocks (bf16), and bias. On-device: out.T tile [o=128, m=1024] accumulates
in PSUM over the 16 kept k-subtiles (k-subtile = 128 rows), with W tiles
stationary and x slabs moving. PSUM is evicted through the scalar engine
with the per-partition bias add fused, then DMA'd out. The host
reassembles the full [8192, 4096] fp32 output.
"""

import sys
import types

import numpy as np
import ml_dtypes

BATCH = 8192
SIZE = 4096
NB = 16
BLOCK = 256
NCORES = 8
MC = BATCH // NCORES  # 1024 rows per core
P = 128
KS = SIZE // P  # 32 k-subtiles
OT = SIZE // P  # 32 o-tiles
MM_N = 512  # moving free dim per matmul

_BUILD_CACHE = {}


def _install_ntff_hook():
    # Register the axon NTFF profiling hook if the image's antenv lacks it.
    if "antenv.axon_hooks" in sys.modules:
        return
    try:
        from trn_agent_boot.trn_boot import _ntff_profile_via_ctypes

        hook = _ntff_profile_via_ctypes("/opt/axon/libaxon_pjrt.so")
        mod = types.ModuleType("antenv.axon_hooks")
        mod.get_axon_ntff_profile_hook = lambda: hook
        sys.modules["antenv.axon_hooks"] = mod
    except Exception:
        pass


def _block_keep_from_mask(mask):
    """Return [NB, NB] bool of kept blocks if mask is block-constant, else None."""
    m4 = np.asarray(mask).reshape(NB, BLOCK, NB, BLOCK)
    keep = m4[:, 0, :, 0]
    uniform = np.all(m4 == keep[:, None, :, None])
    return keep if uniform else None


def _ks_lists(keep):
    """Per o-tile (128 outputs) list of kept k-subtile indices, padded to
    a uniform length (padding points at subtile 0 with zero weights)."""
    lists = []
    for t in range(OT):
        i = (t * P) // BLOCK  # o-block row
        ks = []
        for j in range(NB):
            if keep[i, j]:
                base = (j * BLOCK) // P
                ks.extend(range(base, base + BLOCK // P))
        lists.append(ks)
    n_sub = max(len(l) for l in lists)
    padded = tuple(tuple(l + [-1] * (n_sub - len(l))) for l in lists)
    return padded, n_sub


def _build(ks_lists, n_sub):
    import concourse.mybir as mybir
    import concourse.tile as tile
    from concourse import bacc

    bf16, f32 = mybir.dt.bfloat16, mybir.dt.float32
    nc = bacc.Bacc("TRN2", target_bir_lowering=False)
    xt_d = nc.declare_dram_parameter("xt", [P, KS, MC], bf16, isOutput=False)
    wt_d = nc.declare_dram_parameter("wt", [OT, P, n_sub, P], bf16, isOutput=False)
    bias_d = nc.declare_dram_parameter("biast", [P, OT], f32, isOutput=False)
    out_d = nc.declare_dram_parameter("out", [OT, P, MC], f32, isOutput=True)

    # x DMA issue order: k-subtiles in order of first use across o-tiles.
    ks_order = []
    for t in range(OT):
        for ks in ks_lists[t]:
            if ks >= 0 and ks not in ks_order:
                ks_order.append(ks)
    for ks in range(KS):
        if ks not in ks_order:
            ks_order.append(ks)

    with tile.TileContext(nc) as tc:
        with (
            tc.tile_pool(name="const", bufs=1) as const_pool,
            tc.tile_pool(name="xpool", bufs=1) as xpool,
            tc.tile_pool(name="wpool", bufs=4) as wpool,
            tc.tile_pool(name="opool", bufs=3) as opool,
            tc.tile_pool(name="psum", bufs=2, space="PSUM") as psum_pool,
        ):
            bias_tile = const_pool.tile([P, OT], f32)
            nc.sync.dma_start(out=bias_tile[:], in_=bias_d[:])
            x_tile = xpool.tile([P, KS, MC], bf16)
            for ks in ks_order:
                nc.sync.dma_start(out=x_tile[:, ks, :], in_=xt_d[:, ks, :])
            for t in range(OT):
                w_tile = wpool.tile([P, n_sub, P], bf16, name="w_tile")
                nc.sync.dma_start(out=w_tile[:], in_=wt_d[t])
                ps = psum_pool.tile([P, MC], f32, name="ps")
                for s in range(n_sub):
                    ks = ks_lists[t][s]
                    src = max(ks, 0)  # padded entries multiply zero weights
                    for h in range(MC // MM_N):
                        nc.tensor.matmul(
                            ps[:, h * MM_N : (h + 1) * MM_N],
                            lhsT=w_tile[:, s, :],
                            rhs=x_tile[:, src, h * MM_N : (h + 1) * MM_N],
                            start=(s == 0),
                            stop=(s == n_sub - 1),
                        )
                o_tile = opool.tile([P, MC], f32, name="o_tile")
                nc.vector.tensor_scalar_add(o_tile[:], ps[:], bias_tile[:, t : t + 1])
                nc.sync.dma_start(out=out_d[t], in_=o_tile[:])
    nc.compile()
    return nc


def _get_kernel(ks_lists, n_sub):
    key = (ks_lists, n_sub)
    if key not in _BUILD_CACHE:
        _BUILD_CACHE[key] = _build(ks_lists, n_sub)
    return _BUILD_CACHE[key]


def kernel(x, weight, bias, mask, _trace=False):
    from concourse.bass_utils import run_bass_kernel_spmd

    _install_ntff_hook()

    x = np.asarray(x)
    weight = np.asarray(weight)
    bias = np.asarray(bias, dtype=np.float32)
    keep = _block_keep_from_mask(mask)
    if keep is None:
        # Mask not block-constant: fall back to a dense schedule with the
        # element-masked weights and every k-subtile kept.
        weight = np.where(np.asarray(mask), weight, 0.0).astype(np.float32)
        keep = np.ones((NB, NB), dtype=bool)
    ks_lists, n_sub = _ks_lists(keep)

    nc = _get_kernel(ks_lists, n_sub)

    # Pack weights: wt[t, p, s, q] = W[t*P + q, ks*P + p] for kept subtile ks.
    w4 = weight.reshape(OT, P, KS, P)  # [t, q, ks, p]
    wt = np.zeros((OT, P, n_sub, P), dtype=ml_dtypes.bfloat16)
    for t in range(OT):
        idx = [ks for ks in ks_lists[t]]
        valid = [s for s, ks in enumerate(idx) if ks >= 0]
        sel = w4[t][:, [idx[s] for s in valid], :]  # [q, s_valid, p]
        wt[t][:, valid, :] = sel.transpose(2, 1, 0).astype(ml_dtypes.bfloat16)

    biast = np.ascontiguousarray(
        bias.reshape(OT, P).T, dtype=np.float32
    )  # [P, OT]

    in_maps = []
    for c in range(NCORES):
        xc = x[c * MC : (c + 1) * MC, :]  # [MC, SIZE] fp32
        xt = np.ascontiguousarray(
            xc.reshape(MC, KS, P).transpose(2, 1, 0)
        ).astype(ml_dtypes.bfloat16)  # [P, KS, MC]
        in_maps.append({"xt": xt, "wt": wt, "biast": biast})

    res = run_bass_kernel_spmd(nc, in_maps, list(range(NCORES)), trace=_trace)

    out = np.empty((BATCH, SIZE), dtype=np.float32)
    for c in range(NCORES):
        o = res.results[c]["out"]  # [OT, P, MC]
        out[c * MC : (c + 1) * MC, :] = o.reshape(SIZE, MC).T
    if _trace:
        return out, res
    return out


# revision 3
# speedup vs baseline: 1.0664x; 1.0664x over previous
"""Block-sparse linear layer (x @ (mask*W).T + bias) on 8 TRN2 NeuronCores.

Strategy: data-parallel over batch rows. Each core gets 1024 rows of x
(transposed to [k, m] on host, cast to bf16), the packed kept weight
blocks (bf16), and bias. On-device: out.T tile [o=128, m=1024] accumulates
in PSUM over the 16 kept k-subtiles (k-subtile = 128 rows), with W tiles
stationary and x slabs moving. PSUM is evicted through the scalar engine
with the per-partition bias add fused, then DMA'd out. The host
reassembles the full [8192, 4096] fp32 output.
"""

import sys
import types

import numpy as np
import ml_dtypes

BATCH = 8192
SIZE = 4096
NB = 16
BLOCK = 256
NCORES = 8
MC = BATCH // NCORES  # 1024 rows per core
P = 128
KS = SIZE // P  # 32 k-subtiles
OT = SIZE // P  # 32 o-tiles
MM_N = 512  # moving free dim per matmul

_BUILD_CACHE = {}


def _install_ntff_hook():
    # Register the axon NTFF profiling hook if the image's antenv lacks it.
    if "antenv.axon_hooks" in sys.modules:
        return
    try:
        from trn_agent_boot.trn_boot import _ntff_profile_via_ctypes

        hook = _ntff_profile_via_ctypes("/opt/axon/libaxon_pjrt.so")
        mod = types.ModuleType("antenv.axon_hooks")
        mod.get_axon_ntff_profile_hook = lambda: hook
        sys.modules["antenv.axon_hooks"] = mod
    except Exception:
        pass


def _block_keep_from_mask(mask):
    """Return [NB, NB] bool of kept blocks if mask is block-constant, else None."""
    m4 = np.asarray(mask).reshape(NB, BLOCK, NB, BLOCK)
    keep = m4[:, 0, :, 0]
    uniform = np.all(m4 == keep[:, None, :, None])
    return keep if uniform else None


def _ks_lists(keep):
    """Per o-tile (128 outputs) list of kept k-subtile indices, padded to
    a uniform length (padding points at subtile 0 with zero weights)."""
    lists = []
    for t in range(OT):
        i = (t * P) // BLOCK  # o-block row
        ks = []
        for j in range(NB):
            if keep[i, j]:
                base = (j * BLOCK) // P
                ks.extend(range(base, base + BLOCK // P))
        lists.append(ks)
    n_sub = max(len(l) for l in lists)
    padded = tuple(tuple(l + [-1] * (n_sub - len(l))) for l in lists)
    return padded, n_sub


def _build(ks_lists, n_sub):
    import concourse.mybir as mybir
    import concourse.tile as tile
    from concourse import bacc

    bf16, f32 = mybir.dt.bfloat16, mybir.dt.float32
    nc = bacc.Bacc("TRN2", target_bir_lowering=False)
    xt_d = nc.declare_dram_parameter("xt", [P, KS, MC], bf16, isOutput=False)
    wt_d = nc.declare_dram_parameter("wt", [OT, P, n_sub, P], bf16, isOutput=False)
    bias_d = nc.declare_dram_parameter("biast", [P, OT], f32, isOutput=False)
    out_d = nc.declare_dram_parameter("out", [OT, P, MC], f32, isOutput=True)

    # x DMA issue order: k-subtiles in order of first use across o-tiles.
    ks_order = []
    for t in range(OT):
        for ks in ks_lists[t]:
            if ks >= 0 and ks not in ks_order:
                ks_order.append(ks)
    for ks in range(KS):
        if ks not in ks_order:
            ks_order.append(ks)

    W_BUFS = 4

    with tile.TileContext(nc) as tc:
        with (
            tc.tile_pool(name="const", bufs=1) as const_pool,
            tc.tile_pool(name="xpool", bufs=1) as xpool,
            tc.tile_pool(name="wpool", bufs=W_BUFS) as wpool,
            tc.tile_pool(name="opool", bufs=3) as opool,
            tc.tile_pool(name="psum", bufs=4, space="PSUM") as psum_pool,
        ):
            bias_tile = const_pool.tile([P, OT], f32)
            # W/bias descriptors on the GpSimd queue, x/out on Sync: the
            # per-DMA descriptor generation (~0.6us) serializes per engine,
            # and the PE can't start until W0 has landed.
            nc.gpsimd.dma_start(out=bias_tile[:], in_=bias_d[:])

            w_tiles = {}

            def w_dma(t):
                w = wpool.tile([P, n_sub, P], bf16, name="w_tile")
                nc.gpsimd.dma_start(out=w[:], in_=wt_d[t])
                w_tiles[t] = w

            x_tiles = {}

            def x_dma(ks):
                xk = xpool.tile([P, MC], bf16, name=f"x_{ks}", uniquify=False)
                nc.sync.dma_start(out=xk[:], in_=xt_d[:, ks, :])
                x_tiles[ks] = xk

            # Prefetch the first W_BUFS weight tiles and all x chunks, in
            # consumption order.
            for t in range(W_BUFS):
                w_dma(t)
            for ks in ks_order:
                x_dma(ks)

            for t in range(OT):
                if t >= W_BUFS:
                    w_dma(t)
                w_tile = w_tiles[t]
                ps = psum_pool.tile([P, MC], f32, name="ps")
                for s in range(n_sub):
                    ks = ks_lists[t][s]
                    src = max(ks, 0)  # padded entries multiply zero weights
                    for h in range(MC // MM_N):
                        nc.tensor.matmul(
                            ps[:, h * MM_N : (h + 1) * MM_N],
                            lhsT=w_tile[:, s, :],
                            rhs=x_tiles[src][:, h * MM_N : (h + 1) * MM_N],
                            start=(s == 0),
                            stop=(s == n_sub - 1),
                        )
                o_tile = opool.tile([P, MC], f32, name="o_tile")
                nc.vector.tensor_scalar_add(o_tile[:], ps[:], bias_tile[:, t : t + 1])
                nc.sync.dma_start(out=out_d[t], in_=o_tile[:])
    nc.compile()
    return nc


def _get_kernel(ks_lists, n_sub):
    key = (ks_lists, n_sub)
    if key not in _BUILD_CACHE:
        _BUILD_CACHE[key] = _build(ks_lists, n_sub)
    return _BUILD_CACHE[key]


def kernel(x, weight, bias, mask, _trace=False):
    from concourse.bass_utils import run_bass_kernel_spmd

    _install_ntff_hook()

    x = np.asarray(x)
    weight = np.asarray(weight)
    bias = np.asarray(bias, dtype=np.float32)
    keep = _block_keep_from_mask(mask)
    if keep is None:
        # Mask not block-constant: fall back to a dense schedule with the
        # element-masked weights and every k-subtile kept.
        weight = np.where(np.asarray(mask), weight, 0.0).astype(np.float32)
        keep = np.ones((NB, NB), dtype=bool)
    ks_lists, n_sub = _ks_lists(keep)

    nc = _get_kernel(ks_lists, n_sub)

    # Pack weights: wt[t, p, s, q] = W[t*P + q, ks*P + p] for kept subtile ks.
    w4 = weight.reshape(OT, P, KS, P)  # [t, q, ks, p]
    wt = np.zeros((OT, P, n_sub, P), dtype=ml_dtypes.bfloat16)
    for t in range(OT):
        idx = [ks for ks in ks_lists[t]]
        valid = [s for s, ks in enumerate(idx) if ks >= 0]
        sel = w4[t][:, [idx[s] for s in valid], :]  # [q, s_valid, p]
        wt[t][:, valid, :] = sel.transpose(2, 1, 0).astype(ml_dtypes.bfloat16)

    biast = np.ascontiguousarray(
        bias.reshape(OT, P).T, dtype=np.float32
    )  # [P, OT]

    in_maps = []
    for c in range(NCORES):
        xc = x[c * MC : (c + 1) * MC, :]  # [MC, SIZE] fp32
        xt = np.ascontiguousarray(
            xc.reshape(MC, KS, P).transpose(2, 1, 0)
        ).astype(ml_dtypes.bfloat16)  # [P, KS, MC]
        in_maps.append({"xt": xt, "wt": wt, "biast": biast})

    res = run_bass_kernel_spmd(nc, in_maps, list(range(NCORES)), trace=_trace)

    out = np.empty((BATCH, SIZE), dtype=np.float32)
    for c in range(NCORES):
        o = res.results[c]["out"]  # [OT, P, MC]
        out[c * MC : (c + 1) * MC, :] = o.reshape(SIZE, MC).T
    if _trace:
        return out, res
    return out


# revision 4
# speedup vs baseline: 1.0760x; 1.0090x over previous
"""Block-sparse linear layer (x @ (mask*W).T + bias) on 8 TRN2 NeuronCores.

Strategy: data-parallel over batch rows. Each core gets 1024 rows of x
(transposed to [k, m] on host, cast to bf16), the packed kept weight
blocks (bf16), and bias. On-device: out.T tile [o=128, m=1024] accumulates
in PSUM over the 16 kept k-subtiles (k-subtile = 128 rows), with W tiles
stationary and x slabs moving. PSUM is evicted through the scalar engine
with the per-partition bias add fused, then DMA'd out. The host
reassembles the full [8192, 4096] fp32 output.
"""

import sys
import types

import numpy as np
import ml_dtypes

BATCH = 8192
SIZE = 4096
NB = 16
BLOCK = 256
NCORES = 8
MC = BATCH // NCORES  # 1024 rows per core
P = 128
KS = SIZE // P  # 32 k-subtiles
OT = SIZE // P  # 32 o-tiles
MM_N = 512  # moving free dim per matmul

_BUILD_CACHE = {}


def _install_ntff_hook():
    # Register the axon NTFF profiling hook if the image's antenv lacks it.
    if "antenv.axon_hooks" in sys.modules:
        return
    try:
        from trn_agent_boot.trn_boot import _ntff_profile_via_ctypes

        hook = _ntff_profile_via_ctypes("/opt/axon/libaxon_pjrt.so")
        mod = types.ModuleType("antenv.axon_hooks")
        mod.get_axon_ntff_profile_hook = lambda: hook
        sys.modules["antenv.axon_hooks"] = mod
    except Exception:
        pass


def _block_keep_from_mask(mask):
    """Return [NB, NB] bool of kept blocks if mask is block-constant, else None."""
    m4 = np.asarray(mask).reshape(NB, BLOCK, NB, BLOCK)
    keep = m4[:, 0, :, 0]
    uniform = np.all(m4 == keep[:, None, :, None])
    return keep if uniform else None


def _ks_lists(keep):
    """Per o-tile (128 outputs) list of kept k-subtile indices, padded to
    a uniform length (padding points at subtile 0 with zero weights)."""
    lists = []
    for t in range(OT):
        i = (t * P) // BLOCK  # o-block row
        ks = []
        for j in range(NB):
            if keep[i, j]:
                base = (j * BLOCK) // P
                ks.extend(range(base, base + BLOCK // P))
        lists.append(ks)
    n_sub = max(len(l) for l in lists)
    padded = tuple(tuple(l + [-1] * (n_sub - len(l))) for l in lists)
    return padded, n_sub


def _build(ks_lists, n_sub):
    import concourse.mybir as mybir
    import concourse.tile as tile
    from concourse import bacc

    bf16, f32 = mybir.dt.bfloat16, mybir.dt.float32
    nc = bacc.Bacc("TRN2", target_bir_lowering=False)
    xt_d = nc.declare_dram_parameter("xt", [P, KS, MC], bf16, isOutput=False)
    wt_d = nc.declare_dram_parameter("wt", [OT, P, n_sub, P], bf16, isOutput=False)
    bias_d = nc.declare_dram_parameter("biast", [P, OT], f32, isOutput=False)
    out_d = nc.declare_dram_parameter("out", [OT, P, MC], f32, isOutput=True)

    # x DMA issue order: k-subtiles in order of first use across o-tiles.
    ks_order = []
    for t in range(OT):
        for ks in ks_lists[t]:
            if ks >= 0 and ks not in ks_order:
                ks_order.append(ks)
    for ks in range(KS):
        if ks not in ks_order:
            ks_order.append(ks)

    W_BUFS = 4
    XG = 2  # x chunks per DMA group

    with tile.TileContext(nc) as tc:
        with (
            tc.tile_pool(name="const", bufs=1) as const_pool,
            tc.tile_pool(name="xpool", bufs=1) as xpool,
            tc.tile_pool(name="wpool", bufs=W_BUFS) as wpool,
            tc.tile_pool(name="opool", bufs=3) as opool,
            tc.tile_pool(name="psum", bufs=2, space="PSUM") as psum_pool,
        ):
            bias_tile = const_pool.tile([P, OT], f32)
            # W/bias descriptors on the GpSimd queue, x/out on Sync: the
            # per-DMA descriptor generation (~0.6us) serializes per engine,
            # and the PE can't start until W0 has landed.
            nc.gpsimd.dma_start(out=bias_tile[:], in_=bias_d[:])

            w_tiles = {}

            def w_dma(t):
                w = wpool.tile([P, n_sub, P], bf16, name="w_tile")
                nc.gpsimd.dma_start(out=w[:], in_=wt_d[t])
                w_tiles[t] = w

            # x chunks grouped in pairs that are contiguous in DRAM, issued
            # in consumption order; per-group dependency granularity.
            x_ap = {}
            for t in range(W_BUFS):
                w_dma(t)
            gi = 0
            for gstart in range(0, len(ks_order), XG):
                grp = ks_order[gstart : gstart + XG]
                lo = min(grp)
                assert grp == list(range(lo, lo + len(grp))), grp
                xg = xpool.tile(
                    [P, len(grp), MC], bf16, name=f"x_g{gi}", uniquify=False
                )
                nc.sync.dma_start(out=xg[:], in_=xt_d[:, lo : lo + len(grp), :])
                for off, ks in enumerate(range(lo, lo + len(grp))):
                    x_ap[ks] = xg[:, off, :]
                gi += 1

            for t in range(OT):
                if t >= W_BUFS:
                    w_dma(t)
                w_tile = w_tiles[t]
                ps = psum_pool.tile([P, MC], f32, name="ps")
                for s in range(n_sub):
                    ks = ks_lists[t][s]
                    src = max(ks, 0)  # padded entries multiply zero weights
                    for h in range(MC // MM_N):
                        nc.tensor.matmul(
                            ps[:, h * MM_N : (h + 1) * MM_N],
                            lhsT=w_tile[:, s, :],
                            rhs=x_ap[src][:, h * MM_N : (h + 1) * MM_N],
                            start=(s == 0),
                            stop=(s == n_sub - 1),
                        )
                # Evict in halves so the out-DMA of the first half overlaps
                # the bias-add of the second, shortening the kernel tail.
                o_tile = opool.tile([P, MC], f32, name="o_tile")
                half = MC // 2
                for h in range(2):
                    sl = slice(h * half, (h + 1) * half)
                    nc.vector.tensor_scalar_add(
                        o_tile[:, sl], ps[:, sl], bias_tile[:, t : t + 1]
                    )
                    nc.sync.dma_start(out=out_d[t, :, sl], in_=o_tile[:, sl])
    nc.compile()
    return nc


def _get_kernel(ks_lists, n_sub):
    key = (ks_lists, n_sub)
    if key not in _BUILD_CACHE:
        _BUILD_CACHE[key] = _build(ks_lists, n_sub)
    return _BUILD_CACHE[key]


def kernel(x, weight, bias, mask, _trace=False):
    from concourse.bass_utils import run_bass_kernel_spmd

    _install_ntff_hook()

    x = np.asarray(x)
    weight = np.asarray(weight)
    bias = np.asarray(bias, dtype=np.float32)
    keep = _block_keep_from_mask(mask)
    if keep is None:
        # Mask not block-constant: fall back to a dense schedule with the
        # element-masked weights and every k-subtile kept.
        weight = np.where(np.asarray(mask), weight, 0.0).astype(np.float32)
        keep = np.ones((NB, NB), dtype=bool)
    ks_lists, n_sub = _ks_lists(keep)

    nc = _get_kernel(ks_lists, n_sub)

    # Pack weights: wt[t, p, s, q] = W[t*P + q, ks*P + p] for kept subtile ks.
    w4 = weight.reshape(OT, P, KS, P)  # [t, q, ks, p]
    wt = np.zeros((OT, P, n_sub, P), dtype=ml_dtypes.bfloat16)
    for t in range(OT):
        idx = [ks for ks in ks_lists[t]]
        valid = [s for s, ks in enumerate(idx) if ks >= 0]
        sel = w4[t][:, [idx[s] for s in valid], :]  # [q, s_valid, p]
        wt[t][:, valid, :] = sel.transpose(2, 1, 0).astype(ml_dtypes.bfloat16)

    biast = np.ascontiguousarray(
        bias.reshape(OT, P).T, dtype=np.float32
    )  # [P, OT]

    in_maps = []
    for c in range(NCORES):
        xc = x[c * MC : (c + 1) * MC, :]  # [MC, SIZE] fp32
        xt = np.ascontiguousarray(
            xc.reshape(MC, KS, P).transpose(2, 1, 0)
        ).astype(ml_dtypes.bfloat16)  # [P, KS, MC]
        in_maps.append({"xt": xt, "wt": wt, "biast": biast})

    res = run_bass_kernel_spmd(nc, in_maps, list(range(NCORES)), trace=_trace)

    out = np.empty((BATCH, SIZE), dtype=np.float32)
    for c in range(NCORES):
        o = res.results[c]["out"]  # [OT, P, MC]
        out[c * MC : (c + 1) * MC, :] = o.reshape(SIZE, MC).T
    if _trace:
        return out, res
    return out
